# revision 7
# baseline (speedup 1.0000x reference)
"""Trainium2 Bass kernel for nn_Disc_53515292508892 (ragged_sequence).

Computes: src-GRU (H=1024) over ragged [128,64] token batch -> final hidden,
tgt-GRU seeded with it, then a 2-layer head -> logits [64, 2].
(The reference's ref-encoder outputs are dead code -- skipped.)

Sharding: data-parallel over batch, B=64 -> 8 sequences per NeuronCore,
GRU weights replicated, no inter-core communication.

v3 design (single 8-lane chain, block-pipelined):
  - One recurrence chain per core (8 lanes) so the Whh weight stream runs
    once per step (~2.9us warm).  The serial gate chain is hidden by
    splitting hidden units into blocks A (u<128 per group) and B (u>=128):
    block A's gates/update/transpose run on DVE/ACT while the PE still
    streams block B's columns and the next step's A-input k-tiles.
  - k-tiles that contract against block B's hidden units (k>=4) are
    deferred to the end of each PSUM group so the PE only needs hTb late.
  - xw preacts folded into PSUM via K=128 identity-column matmuls
    (ident[:, 32j:32j+8] selects the 8 batch rows); bhh_n likewise.
  - hTa/hTb (matmul lhsT) via DVE 32x32-block transpose SBUF->SBUF; the
    Whh k-dim row permutation makes block-transposed h a valid lhsT:
    k-tile c, partition p=32J+x  <->  hidden unit 256J+32c+x.
  - Gate column order per quadrant J: [rA zA rB zB nA nB] (128 each);
    z columns sign-flipped so z' = 1-z = sigmoid(-pre_z).

Perf ledger (HW exec):
  2128179 ns  v1 baseline (single chain, unpipelined, HAM-cold streams)
  1798774 ns  v2 two interleaved 4-lane chains (PE dense+warm but the
              batch split doubles the weight-stream work)
"""

import sys
import functools

sys.path.insert(0, "/opt/trn_rl_repo")

import numpy as np
import concourse.mybir as mybir
from concourse import bacc, tile
from concourse.bass_utils import run_bass_kernel_spmd

f16 = mybir.dt.float16
f32 = mybir.dt.float32
AO = mybir.AluOpType
AF = mybir.ActivationFunctionType

V, D, H = 32000, 512, 1024
T = 128          # steps per GRU (T_SRC = T_TGT = 128)
BL = 8           # batch lanes per core
NCORES = 8
NG = 4           # col-tile quadrants
GW = 768         # gate columns per quadrant (rA zA rB zB nA nB; 128 each)
KT = H // 128    # 8 k-tiles over hidden
KD = D // 128    # 4 k-tiles over embedding dim

GPSIMD_DE = True   # run d=n-h / e=d*z' on GPSIMD to unload DVE


# ----------------------------------------------------------------------------
# host-side weight/layout prep
# ----------------------------------------------------------------------------

def _col_perm():
    """newcol g in [0,3072): quadrant j=g//768; within-quadrant layout
    [rA zA rB zB nA nB] each 128 wide.  Returns orig-row, sign, is_n."""
    g = np.arange(3 * H)
    j = g // GW
    rem = g % GW
    blk = rem // 128          # 0:rA 1:zA 2:rB 3:zB 4:nA 5:nB
    q = rem % 128
    gate = np.where(blk < 4, blk % 2, 2)          # 0=r 1=z 2=n
    u = np.where(blk < 4, (blk // 2) * 128 + q,   # rz: A->u<128, B->u>=128
                 (blk - 4) * 128 + q)             # n:  nA->u<128, nB->u>=128
    row = gate * H + 256 * j + u
    sign = np.where(gate == 1, -1.0, 1.0).astype(np.float32)
    is_n = gate == 2
    return row, sign, is_n


def _k_unit(c):
    """hidden unit held by partition p for k-tile c: 256*(p//32)+32c+(p%32)."""
    p = np.arange(128)
    return 256 * (p // 32) + 32 * c + (p % 32)


def _prep_shared(inputs, n_steps):
    row, sign, is_n = _col_perm()
    out = {}
    for g, wih, whh, bih, bhh in (
        ("src", inputs["src_Wih"], inputs["src_Whh"], inputs["src_bih"], inputs["src_bhh"]),
        ("tgt", inputs["tgt_Wih"], inputs["tgt_Whh"], inputs["tgt_bih"], inputs["tgt_bhh"]),
    ):
        wcol = whh[row] * sign[:, None]                       # [3072, 1024]
        whh_a = np.empty((KT, 128, 3 * H), np.float16)
        for c in range(KT):
            whh_a[c] = wcol[:, _k_unit(c)].T.astype(np.float16)
        out[f"whh_{g}"] = np.ascontiguousarray(whh_a)
        wih_a = (wih[row] * sign[:, None]).T.astype(np.float16)   # [512, 3072]
        out[f"wih_{g}"] = np.ascontiguousarray(wih_a.reshape(KD, 128, 3 * H))
        bias_vec = sign * bih[row] + sign * np.where(is_n, 0.0, bhh[row])
        out[f"bias_{g}"] = np.broadcast_to(
            bias_vec.astype(np.float16), (128, 3 * H)).copy()
        # bhh_n broadcast strip for the K=128 identity fold: rows 32j+b, col u
        bhhn = np.zeros((128, 256), np.float16)
        for j in range(NG):
            bhhn[32 * j:32 * j + 32, :] = bhh[2 * H + 256 * j: 2 * H + 256 * (j + 1)].astype(np.float16)
        out[f"bhhn_{g}"] = bhhn
    p1 = np.empty((128, KT * 64), np.float32)
    for c in range(KT):
        p1[:, 64 * c:64 * (c + 1)] = inputs["p1_W"][:, _k_unit(c)].T
    out["p1T"] = p1.astype(np.float16)
    out["p1b"] = np.broadcast_to(inputs["p1_b"].astype(np.float16), (128, 64)).copy()
    out["p2T"] = inputs["p2_W"].T.astype(np.float16)                  # [64, 2]
    out["p2b"] = np.broadcast_to(inputs["p2_b"].astype(np.float32), (128, 2)).copy()
    out["ident"] = np.eye(128, dtype=np.float16)
    return out


def _prep_core(inputs, emb16, core, n_steps):
    """Per-core tensors: gathered/transposed embeddings and masks."""
    sl = slice(BL * core, BL * (core + 1))
    out = {}
    for g, ids_key in (("src", "src"), ("tgt", "tgt")):
        ids = np.asarray(inputs[ids_key])[:n_steps, sl]           # [T, 8]
        x = emb16[ids]                                            # [T, 8, 512]
        out[f"xT_{g}"] = np.ascontiguousarray(
            x.transpose(2, 0, 1).reshape(KD, 128, n_steps * BL))
    masks = np.zeros((128, 2 * n_steps), np.float32)
    for gi, len_key in enumerate(("src_lengths", "tgt_lengths")):
        ln = np.asarray(inputs[len_key])[sl]                      # [8]
        t = np.arange(n_steps)
        m = (t[None, :] < ln[:, None]).astype(np.float32)         # [8, T]
        for j in range(NG):
            masks[32 * j:32 * j + BL, gi * n_steps:(gi + 1) * n_steps] = m
    out["masks"] = masks
    return out


# ----------------------------------------------------------------------------
# device program
# ----------------------------------------------------------------------------

def build_program(n_steps=T):
    nc = bacc.Bacc("TRN2", target_bir_lowering=False, debug=False,
                   num_devices=NCORES)
    TB = n_steps * BL    # 1024 xw rows per GRU

    dp = nc.declare_dram_parameter
    d_xT = {g: dp(f"xT_{g}", [KD, 128, TB], f16, isOutput=False) for g in ("src", "tgt")}
    d_whh = {g: dp(f"whh_{g}", [KT, 128, 3 * H], f16, isOutput=False) for g in ("src", "tgt")}
    d_wih = {g: dp(f"wih_{g}", [KD, 128, 3 * H], f16, isOutput=False) for g in ("src", "tgt")}
    d_bias = {g: dp(f"bias_{g}", [128, 3 * H], f16, isOutput=False) for g in ("src", "tgt")}
    d_bhhn = {g: dp(f"bhhn_{g}", [128, 256], f16, isOutput=False) for g in ("src", "tgt")}
    d_masks = dp("masks", [128, 2 * n_steps], f32, isOutput=False)
    d_p1T = dp("p1T", [128, KT * 64], f16, isOutput=False)
    d_p1b = dp("p1b", [128, 64], f16, isOutput=False)
    d_p2T = dp("p2T", [64, 2], f16, isOutput=False)
    d_p2b = dp("p2b", [128, 2], f32, isOutput=False)
    d_ident = dp("ident", [128, 128], f16, isOutput=False)
    d_logits = dp("logits", [BL, 2], f32, isOutput=True)

    with tile.TileContext(nc) as tc:
        with tc.tile_pool(name="const", bufs=1) as cpool, \
             tc.tile_pool(name="work", bufs=2) as wpool, \
             tc.tile_pool(name="xwload", bufs=4) as xwpool, \
             tc.tile_pool(name="p1ev", bufs=4) as evpool, \
             tc.tile_pool(name="psum", bufs=1, space="PSUM") as psum, \
             tc.tile_pool(name="dram", bufs=1, space="DRAM") as dram:

            # ---- resident constants -------------------------------------
            whh_sb, xT_sb, bias_sb, bhhn_sb = {}, {}, {}, {}
            for g in ("src", "tgt"):
                whh_sb[g] = cpool.tile([128, KT * 3 * H], f16, tag=f"whh_{g}", name=f"whh_{g}")
                for k in range(KT):
                    nc.sync.dma_start(whh_sb[g][:, 3 * H * k:3 * H * (k + 1)], d_whh[g][k])
                xT_sb[g] = cpool.tile([128, KD * TB], f16, tag=f"xT_{g}", name=f"xT_{g}")
                for k in range(KD):
                    nc.sync.dma_start(xT_sb[g][:, TB * k:TB * (k + 1)], d_xT[g][k])
                bias_sb[g] = cpool.tile([128, 3 * H], f16, tag=f"bias_{g}", name=f"biassb_{g}")
                nc.sync.dma_start(bias_sb[g][:], d_bias[g][:])
                bhhn_sb[g] = cpool.tile([128, 256], f16, tag=f"bhhn_{g}", name=f"bhhnsb_{g}")
                nc.sync.dma_start(bhhn_sb[g][:], d_bhhn[g][:])
            masks_sb = cpool.tile([128, 2 * n_steps], f32, tag="masks")
            nc.sync.dma_start(masks_sb[:], d_masks[:])
            ident_sb = cpool.tile([128, 128], f16, tag="ident")
            nc.sync.dma_start(ident_sb[:], d_ident[:])
            p1T_sb = cpool.tile([128, KT * 64], f16, tag="p1T")
            nc.sync.dma_start(p1T_sb[:], d_p1T[:])
            p1b_sb = cpool.tile([128, 64], f16, tag="p1b")
            nc.sync.dma_start(p1b_sb[:], d_p1b[:])
            p2T_sb = cpool.tile([64, 2], f16, tag="p2T")
            nc.sync.dma_start(p2T_sb[:], d_p2T[:])
            p2b_sb = cpool.tile([128, 2], f32, tag="p2b")
            nc.sync.dma_start(p2b_sb[:], d_p2b[:])

            # ---- phase 1: xw = x @ Wih.T + bias -> DRAM -----------------
            xw_dram = {}
            for g in ("src", "tgt"):
                xw_dram[g] = dram.tile([TB, 3 * H], f16, tag=f"xw_{g}", name=f"xwdram_{g}")
            n_mstrip = TB // 128  # 8

            for g in ("src", "tgt"):
                for chk in range(6):
                    wihs = []
                    for kd in range(KD):
                        wt = evpool.tile([128, 512], f16, tag="wih_s", name="wih_s",
                                         bufs=8)
                        nc.sync.dma_start(
                            wt[:], d_wih[g][kd, :, 512 * chk:512 * (chk + 1)])
                        wihs.append(wt)
                    for mi in range(n_mstrip):
                        m0 = 128 * mi
                        ps = psum.tile([128, 512], f32, tag="p1", name="p1ps", bufs=2)
                        for kd in range(KD):
                            nc.tensor.matmul(
                                ps[:],
                                xT_sb[g][:, TB * kd + m0: TB * kd + m0 + 128],
                                wihs[kd][:],
                                start=(kd == 0), stop=(kd == KD - 1),
                            )
                        ev = evpool.tile([128, 512], f16, tag="ev", name="ev")
                        nc.vector.tensor_add(
                            ev[:], ps[:],
                            bias_sb[g][:, 512 * chk:512 * (chk + 1)])
                        nc.sync.dma_start(
                            xw_dram[g][m0:m0 + 128, 512 * chk:512 * (chk + 1)],
                            ev[:])

            # ---- recurrence ---------------------------------------------
            hA = wpool.tile([128, 128], f16, tag="hA", name="hA0")
            hB = wpool.tile([128, 128], f16, tag="hB", name="hB0")
            hTa = wpool.tile([128, 128], f16, tag="hTa", name="hTa0")
            hTb = wpool.tile([128, 128], f16, tag="hTb", name="hTb0")
            for tl in (hA, hB, hTa, hTb):
                nc.vector.memset(tl[:], 0.0)
            # pre-warm xw buffers: folds multiply unselected partitions by 0
            # and stale SBUF may hold NaN bit patterns
            for _ in range(4):
                warm = xwpool.tile([128, GW], f16, tag="xw_t", name="xww")
                nc.vector.memset(warm[:], 0.0)

            def mm(out_ap, lhsT, rhs, start, stop, j):
                nc.tensor.matmul(out_ap, lhsT, rhs, start=start, stop=stop,
                                 tile_position=(0, 32 * j))

            eng_de = nc.gpsimd if GPSIMD_DE else nc.vector

            for step in range(2 * n_steps):
                g = "src" if step < n_steps else "tgt"
                t = step % n_steps
                mcol = t if g == "src" else n_steps + t

                xw_t = xwpool.tile([128, GW], f16, tag="xw_t", name="xw_t")
                for j in range(NG):
                    nc.sync.dma_start(
                        xw_t[32 * j:32 * j + BL, :],
                        xw_dram[g][t * BL:(t + 1) * BL, GW * j:GW * (j + 1)])

                przA = psum.tile([128, 256], f32, tag="rzA", name="przA")
                przB = psum.tile([128, 256], f32, tag="rzB", name="przB")
                pnA = psum.tile([128, 128], f32, tag="nA", name="pnA")
                pnB = psum.tile([128, 128], f32, tag="nB", name="pnB")

                W = whh_sb[g]

                def kt_mms(ps_tile, coff, width, klo, khi, start, stop):
                    for ki in range(klo, khi):
                        lhsT = (hTa[:, 32 * ki:32 * ki + BL] if ki < 4
                                else hTb[:, 32 * (ki - 4):32 * (ki - 4) + BL])
                        for j in range(NG):
                            mm(ps_tile[32 * j:32 * j + BL, :],
                               lhsT,
                               W[:, 3 * H * ki + GW * j + coff:
                                 3 * H * ki + GW * j + coff + width],
                               start and ki == klo, stop and ki == khi - 1, j)

                # folds + A-input ktiles, then deferred B-input ktiles
                for j in range(NG):
                    mm(przA[32 * j:32 * j + BL, :], ident_sb[:, 32 * j:32 * j + BL],
                       xw_t[:, 0:256], True, False, j)
                kt_mms(przA, 0, 256, 0, 4, False, False)
                for j in range(NG):
                    mm(pnA[32 * j:32 * j + BL, :], ident_sb[:, 32 * j:32 * j + BL],
                       bhhn_sb[g][:, 0:128], True, False, j)
                kt_mms(pnA, 512, 128, 0, 4, False, False)
                kt_mms(przA, 0, 256, 4, 8, False, True)     # needs hTb
                kt_mms(pnA, 512, 128, 4, 8, False, True)
                for j in range(NG):
                    mm(przB[32 * j:32 * j + BL, :], ident_sb[:, 32 * j:32 * j + BL],
                       xw_t[:, 256:512], True, False, j)
                kt_mms(przB, 256, 256, 0, 4, False, False)
                for j in range(NG):
                    mm(pnB[32 * j:32 * j + BL, :], ident_sb[:, 32 * j:32 * j + BL],
                       bhhn_sb[g][:, 128:256], True, False, j)
                kt_mms(pnB, 640, 128, 0, 4, False, False)
                kt_mms(przB, 256, 256, 4, 8, False, True)
                kt_mms(pnB, 640, 128, 4, 8, False, True)

                # gate chains (block A then block B); strip views, 8 valid
                # lanes per 32-partition group
                new_h, new_hT = {}, {}
                for X, prz, pn, h_old, xw_off in (
                    ("A", przA, pnA, hA, 512),
                    ("B", przB, pnB, hB, 640),
                ):
                    rzs = wpool.tile([128, 256], f16, tag=f"rzs{X}", name=f"rzs{X}")
                    nc.scalar.activation(rzs[:], prz[:], AF.Sigmoid)
                    tn2 = wpool.tile([128, 128], f16, tag=f"tn2{X}", name=f"tn2{X}")
                    nc.vector.tensor_mul(tn2[:], pn[:], rzs[:, 0:128])
                    sn = wpool.tile([128, 128], f16, tag=f"sn{X}", name=f"sn{X}")
                    nc.vector.tensor_add(sn[:], tn2[:], xw_t[:, xw_off:xw_off + 128])
                    n_t = wpool.tile([128, 128], f16, tag=f"nt{X}", name=f"nt{X}")
                    nc.scalar.activation(n_t[:], sn[:], AF.Tanh)
                    d_t = wpool.tile([128, 128], f16, tag=f"dt{X}", name=f"dt{X}")
                    eng_de.tensor_sub(d_t[:], n_t[:], h_old[:])
                    e_t = wpool.tile([128, 128], f16, tag=f"et{X}", name=f"et{X}")
                    eng_de.tensor_mul(e_t[:], d_t[:], rzs[:, 128:256])
                    h_new = wpool.tile([128, 128], f16, tag=f"h{X}", name=f"h{X}n")
                    nc.vector.scalar_tensor_tensor(
                        h_new[:], e_t[:], masks_sb[:, mcol:mcol + 1], h_old[:],
                        AO.mult, AO.add)
                    hT_new = wpool.tile([128, 128], f16, tag=f"hT{X.lower()}",
                                        name=f"hT{X.lower()}n")
                    nc.vector.transpose(hT_new[:], h_new[:])
                    new_h[X], new_hT[X] = h_new, hT_new

                hA, hTa = new_h["A"], new_hT["A"]
                hB, hTb = new_h["B"], new_hT["B"]

            # ---- head ----------------------------------------------------
            ph = psum.tile([128, 512], f32, tag="p1", name="ph", bufs=2)
            for ki in range(KT):
                lhsT = (hTa[:, 32 * ki:32 * ki + BL] if ki < 4
                        else hTb[:, 32 * (ki - 4):32 * (ki - 4) + BL])
                nc.tensor.matmul(
                    ph[0:BL, 0:64],
                    lhsT,
                    p1T_sb[:, 64 * ki:64 * (ki + 1)],
                    start=(ki == 0), stop=(ki == KT - 1),
                )
            t1s = wpool.tile([128, 64], f16, tag="t1s")
            nc.vector.tensor_add(t1s[0:BL, :], ph[0:BL, 0:64], p1b_sb[0:BL, :])
            t1 = wpool.tile([128, 64], f16, tag="t1")
            nc.scalar.activation(t1[0:BL, :], t1s[0:BL, :], AF.Tanh)

            pt1 = psum.tile([128, 256], f16, tag="tp", name="pt1")
            nc.tensor.transpose(pt1[0:64, 0:BL], t1[0:BL, 0:64], ident_sb[0:BL, 0:BL])
            t1T = wpool.tile([64, BL], f16, tag="t1T")
            nc.vector.tensor_copy(t1T[:], pt1[0:64, 0:BL])

            pl = psum.tile([128, 512], f32, tag="p1", name="pl", bufs=2)
            nc.tensor.matmul(pl[0:BL, 0:2], t1T[:], p2T_sb[:], start=True, stop=True)
            lg = wpool.tile([128, 2], f32, tag="lg")
            nc.vector.tensor_add(lg[0:BL, :], pl[0:BL, 0:2], p2b_sb[0:BL, :])
            nc.sync.dma_start(d_logits[:], lg[0:BL, :])

    nc.compile()
    return nc


# ----------------------------------------------------------------------------
# entry point
# ----------------------------------------------------------------------------

@functools.lru_cache(maxsize=2)
def _cached_program(n_steps):
    return build_program(n_steps)


def run(inputs, n_steps=T, trace=False):
    inputs = {k: np.asarray(v) for k, v in inputs.items()}
    nc = _cached_program(n_steps)
    shared = _prep_shared(inputs, n_steps)
    emb16 = np.asarray(inputs["emb"]).astype(np.float16)
    in_maps = []
    for c in range(NCORES):
        m = dict(shared)
        m.update(_prep_core(inputs, emb16, c, n_steps))
        in_maps.append(m)
    res = run_bass_kernel_spmd(nc, in_maps, list(range(NCORES)), trace=trace)
    logits = np.concatenate([res.results[c]["logits"] for c in range(NCORES)], axis=0)
    return logits, res


def kernel(**inputs) -> np.ndarray:
    logits, _ = run(inputs)
    return logits.astype(np.float32)


# revision 10
# speedup vs baseline: 1.3197x; 1.3197x over previous
"""Trainium2 Bass kernel for nn_Disc_53515292508892 (ragged_sequence).

Computes: src-GRU (H=1024) over ragged [128,64] token batch -> final hidden,
tgt-GRU seeded with it, then a 2-layer head -> logits [64, 2].
(The reference's ref-encoder outputs are dead code -- skipped.)

Sharding: data-parallel over batch, B=64 -> 8 sequences per NeuronCore,
GRU weights replicated, no inter-core communication.

v2 design (two interleaved chains):
  - The per-step recurrence is latency-bound: weight stream (~2.9us warm)
    plus a serial gate chain (~2.9us) that idles the PE and lets HAM
    re-throttle it to 1.2 GHz.  Fix: split the core's 8 lanes into TWO
    independent 4-lane chains and interleave their steps -- chain 0's gate
    chain runs on DVE/ACT while chain 1's weight stream keeps the PE busy
    (and warm).  PE ~100% occupied; per-step cost -> ~stream time.
  - xw (input-gate preacts incl. biases) folded into PSUM via a K=4
    identity matmul; bhh_n folded via a K=1 ones matmul.  Kills two DVE
    adds per step; sigmoid reads PSUM directly.
  - hT (matmul lhsT layout) produced by DVE 32x32-block transpose
    (nc.vector.transpose) straight SBUF->SBUF.  The Whh k-dim row
    permutation is chosen so block-transposed h IS a valid lhsT:
    k-tile c, partition p=32J+x  <->  hidden unit 256J+32c+x.
  - Gate column order per PE quadrant J: [r(256) | z'(256) | n(256)] for
    hidden group J (units 256J..256J+256); z' columns sign-flipped so
    z' = 1-z = sigmoid(-pre_z).

Per chain-step: PE streams rz (fold + 8 ktiles, N=512/quadrant) then n
(fold + 8 ktiles, N=256/quadrant); chain: sigmoid(psum_rz) -> tn2 =
psum_n*r -> sn = tn2+xw_n -> n = tanh(sn) -> d = n-h -> e = d*z' ->
h' = mask*e + h -> hTa/hTb = block-transpose(h').
"""

import sys
import functools

sys.path.insert(0, "/opt/trn_rl_repo")

import numpy as np
import concourse.mybir as mybir
from concourse import bacc, tile
from concourse.bass_utils import run_bass_kernel_spmd

f16 = mybir.dt.float16
f32 = mybir.dt.float32
AO = mybir.AluOpType
AF = mybir.ActivationFunctionType

V, D, H = 32000, 512, 1024
T = 128          # steps per GRU (T_SRC = T_TGT = 128)
BL = 8           # batch per core
NCH = 2          # interleaved chains per core
NL = BL // NCH   # lanes per chain (4)
NCORES = 8
NG = 4           # col-tile quadrants
GW = 768         # gate columns per quadrant (256 r | 256 z' | 256 n)
KT = H // 128    # 8 k-tiles over hidden
KD = D // 128    # 4 k-tiles over embedding dim


# ----------------------------------------------------------------------------
# host-side weight/layout prep
# ----------------------------------------------------------------------------

def _col_perm():
    """newcol g in [0,3072): quadrant j=g//768, gate=(g%768)//256, u=g%256.
    orig W row = gate_base + 256j + u;  sign=-1 for z columns; is_n mask."""
    g = np.arange(3 * H)
    j = g // GW
    rem = g % GW
    gate = rem // 256
    u = rem % 256
    row = gate * H + 256 * j + u
    sign = np.where(gate == 1, -1.0, 1.0).astype(np.float32)
    is_n = gate == 2
    return row, sign, is_n


def _k_unit(c):
    """hidden unit held by partition p for k-tile c: 256*(p//32)+32c+(p%32)."""
    p = np.arange(128)
    return 256 * (p // 32) + 32 * c + (p % 32)


def _prep_shared(inputs, n_steps):
    row, sign, is_n = _col_perm()
    out = {}
    for g, wih, whh, bih, bhh in (
        ("src", inputs["src_Wih"], inputs["src_Whh"], inputs["src_bih"], inputs["src_bhh"]),
        ("tgt", inputs["tgt_Wih"], inputs["tgt_Whh"], inputs["tgt_bih"], inputs["tgt_bhh"]),
    ):
        wcol = whh[row] * sign[:, None]                       # [3072, 1024]
        whh_a = np.empty((KT, 128, 3 * H), np.float16)
        for c in range(KT):
            whh_a[c] = wcol[:, _k_unit(c)].T.astype(np.float16)
        out[f"whh_{g}"] = np.ascontiguousarray(whh_a)
        wih_a = (wih[row] * sign[:, None]).T.astype(np.float16)   # [512, 3072]
        out[f"wih_{g}"] = np.ascontiguousarray(wih_a.reshape(KD, 128, 3 * H))
        bias_vec = sign * bih[row] + sign * np.where(is_n, 0.0, bhh[row])
        out[f"bias_{g}"] = np.broadcast_to(
            bias_vec.astype(np.float16), (128, 3 * H)).copy()
        # bhh_n broadcast strip for the K=128 identity fold: rows 32j+b
        bhhn = np.zeros((128, 256), np.float16)
        for j in range(NG):
            bhhn[32 * j:32 * j + 32, :] = bhh[2 * H + 256 * j: 2 * H + 256 * (j + 1)].astype(np.float16)
        out[f"bhhn_{g}"] = bhhn
    p1 = np.empty((128, KT * 64), np.float32)
    for c in range(KT):
        p1[:, 64 * c:64 * (c + 1)] = inputs["p1_W"][:, _k_unit(c)].T
    out["p1T"] = p1.astype(np.float16)
    out["p1b"] = np.broadcast_to(inputs["p1_b"].astype(np.float16), (128, 64)).copy()
    out["p2T"] = inputs["p2_W"].T.astype(np.float16)                  # [64, 2]
    out["p2b"] = np.broadcast_to(inputs["p2_b"].astype(np.float32), (128, 2)).copy()
    out["ident"] = np.eye(128, dtype=np.float16)
    return out


def _prep_core(inputs, emb16, core, n_steps):
    """Per-core tensors: per-chain gathered/transposed embeddings and masks."""
    out = {}
    for ch in range(NCH):
        sl = slice(BL * core + NL * ch, BL * core + NL * (ch + 1))
        for g, ids_key in (("src", "src"), ("tgt", "tgt")):
            ids = np.asarray(inputs[ids_key])[:n_steps, sl]           # [T, 4]
            x = emb16[ids]                                            # [T, 4, 512]
            out[f"xT_{g}_{ch}"] = np.ascontiguousarray(
                x.transpose(2, 0, 1).reshape(KD, 128, n_steps * NL))
        masks = np.zeros((128, 2 * n_steps), np.float32)
        for gi, len_key in enumerate(("src_lengths", "tgt_lengths")):
            ln = np.asarray(inputs[len_key])[sl]                      # [4]
            t = np.arange(n_steps)
            m = (t[None, :] < ln[:, None]).astype(np.float32)         # [4, T]
            for j in range(NG):
                masks[32 * j:32 * j + NL, gi * n_steps:(gi + 1) * n_steps] = m
        out[f"masks_{ch}"] = masks
    return out


# ----------------------------------------------------------------------------
# device program
# ----------------------------------------------------------------------------

def build_program(n_steps=T):
    nc = bacc.Bacc("TRN2", target_bir_lowering=False, debug=False,
                   num_devices=NCORES)
    TBc = n_steps * NL   # 512 rows of xw per chain per GRU

    dp = nc.declare_dram_parameter
    d_xT = {(g, c): dp(f"xT_{g}_{c}", [KD, 128, TBc], f16, isOutput=False)
            for g in ("src", "tgt") for c in range(NCH)}
    d_whh = {g: dp(f"whh_{g}", [KT, 128, 3 * H], f16, isOutput=False) for g in ("src", "tgt")}
    d_wih = {g: dp(f"wih_{g}", [KD, 128, 3 * H], f16, isOutput=False) for g in ("src", "tgt")}
    d_bias = {g: dp(f"bias_{g}", [128, 3 * H], f16, isOutput=False) for g in ("src", "tgt")}
    d_bhhn = {g: dp(f"bhhn_{g}", [128, 256], f16, isOutput=False) for g in ("src", "tgt")}
    d_masks = {c: dp(f"masks_{c}", [128, 2 * n_steps], f32, isOutput=False)
               for c in range(NCH)}
    d_p1T = dp("p1T", [128, KT * 64], f16, isOutput=False)
    d_p1b = dp("p1b", [128, 64], f16, isOutput=False)
    d_p2T = dp("p2T", [64, 2], f16, isOutput=False)
    d_p2b = dp("p2b", [128, 2], f32, isOutput=False)
    d_ident = dp("ident", [128, 128], f16, isOutput=False)
    d_logits = dp("logits", [BL, 2], f32, isOutput=True)

    with tile.TileContext(nc) as tc:
        with tc.tile_pool(name="const", bufs=1) as cpool, \
             tc.tile_pool(name="work", bufs=2) as wpool, \
             tc.tile_pool(name="xwload", bufs=4) as xwpool, \
             tc.tile_pool(name="p1ev", bufs=4) as evpool, \
             tc.tile_pool(name="psum", bufs=1, space="PSUM") as psum, \
             tc.tile_pool(name="dram", bufs=1, space="DRAM") as dram:

            # ---- resident constants -------------------------------------
            # phase-1 inputs (xT/bias) load FIRST; the 12.6MB whh preload
            # goes last -- it is only needed when the recurrence starts
            whh_sb, xT_sb, bias_sb, bhhn_sb = {}, {}, {}, {}
            for g in ("src", "tgt"):
                for c in range(NCH):
                    xT_sb[(g, c)] = cpool.tile([128, KD * TBc], f16,
                                               tag=f"xT_{g}_{c}", name=f"xT_{g}_{c}")
                    for k in range(KD):
                        nc.sync.dma_start(
                            xT_sb[(g, c)][:, TBc * k:TBc * (k + 1)], d_xT[(g, c)][k])
                bias_sb[g] = cpool.tile([128, 3 * H], f16, tag=f"bias_{g}", name=f"biassb_{g}")
                nc.sync.dma_start(bias_sb[g][:], d_bias[g][:])
                bhhn_sb[g] = cpool.tile([128, 256], f16, tag=f"bhhn_{g}", name=f"bhhnsb_{g}")
                nc.sync.dma_start(bhhn_sb[g][:], d_bhhn[g][:])
            for g in ("src", "tgt"):
                whh_sb[g] = cpool.tile([128, KT * 3 * H], f16, tag=f"whh_{g}", name=f"whh_{g}")
            masks_sb = {}
            for c in range(NCH):
                masks_sb[c] = cpool.tile([128, 2 * n_steps], f32, tag=f"masks_{c}",
                                         name=f"masks_{c}")
                nc.sync.dma_start(masks_sb[c][:], d_masks[c][:])
            ident_sb = cpool.tile([128, 128], f16, tag="ident")
            nc.sync.dma_start(ident_sb[:], d_ident[:])
            p1T_sb = cpool.tile([128, KT * 64], f16, tag="p1T")
            nc.sync.dma_start(p1T_sb[:], d_p1T[:])
            p1b_sb = cpool.tile([128, 64], f16, tag="p1b")
            nc.sync.dma_start(p1b_sb[:], d_p1b[:])
            p2T_sb = cpool.tile([64, 2], f16, tag="p2T")
            nc.sync.dma_start(p2T_sb[:], d_p2T[:])
            p2b_sb = cpool.tile([128, 2], f32, tag="p2b")
            nc.sync.dma_start(p2b_sb[:], d_p2b[:])
            for g in ("src", "tgt"):
                for k in range(KT):
                    nc.gpsimd.dma_start(whh_sb[g][:, 3 * H * k:3 * H * (k + 1)], d_whh[g][k])

            # ---- phase 1: xw = x @ Wih.T + bias -> DRAM -----------------
            xw_dram = {}
            for g in ("src", "tgt"):
                for c in range(NCH):
                    xw_dram[(g, c)] = dram.tile([TBc, 3 * H], f16,
                                                tag=f"xw_{g}_{c}", name=f"xwdram_{g}_{c}")
            n_mstrip = TBc // 128  # 4

            for g in ("src", "tgt"):
                for chk in range(6):
                    wihs = []
                    for kd in range(KD):
                        wt = evpool.tile([128, 512], f16, tag="wih_s", name="wih_s",
                                         bufs=8)
                        nc.sync.dma_start(
                            wt[:], d_wih[g][kd, :, 512 * chk:512 * (chk + 1)])
                        wihs.append(wt)
                    for c in range(NCH):
                        for mi in range(n_mstrip):
                            m0 = 128 * mi
                            ps = psum.tile([128, 512], f32, tag="p1", name="p1ps", bufs=2)
                            for kd in range(KD):
                                nc.tensor.matmul(
                                    ps[:],
                                    xT_sb[(g, c)][:, TBc * kd + m0: TBc * kd + m0 + 128],
                                    wihs[kd][:],
                                    start=(kd == 0), stop=(kd == KD - 1),
                                )
                            ev = evpool.tile([128, 512], f16, tag="ev", name="ev")
                            nc.vector.tensor_add(
                                ev[:], ps[:],
                                bias_sb[g][:, 512 * chk:512 * (chk + 1)])
                            nc.sync.dma_start(
                                xw_dram[(g, c)][m0:m0 + 128, 512 * chk:512 * (chk + 1)],
                                ev[:])

            # ---- recurrence: two interleaved 4-lane chains --------------
            h_str, hTa, hTb = {}, {}, {}
            for c in range(NCH):
                h_str[c] = wpool.tile([128, 256], f16, tag=f"h_{c}", name=f"h0_{c}")
                hTa[c] = wpool.tile([128, 128], f16, tag=f"hTa_{c}", name=f"hTa0_{c}")
                hTb[c] = wpool.tile([128, 128], f16, tag=f"hTb_{c}", name=f"hTb0_{c}")
                nc.vector.memset(h_str[c][:], 0.0)
                nc.vector.memset(hTa[c][:], 0.0)
                nc.vector.memset(hTb[c][:], 0.0)

            for c in range(NCH):
                for _ in range(4):
                    warm = xwpool.tile([128, GW], f16, tag=f"xw_{c}", name=f"xww_{c}")
                    nc.vector.memset(warm[:], 0.0)

            for step in range(2 * n_steps):
                g = "src" if step < n_steps else "tgt"
                t = step % n_steps
                mcol = t if g == "src" else n_steps + t

                for c in range(NCH):
                    xw_t = xwpool.tile([128, GW], f16, tag=f"xw_{c}", name=f"xw_{c}")
                    for j in range(NG):
                        nc.sync.dma_start(
                            xw_t[32 * j:32 * j + NL, :],
                            xw_dram[(g, c)][t * NL:(t + 1) * NL, GW * j:GW * (j + 1)])

                    pmm_rz = psum.tile([128, 512], f32, tag=f"rz_{c}", name=f"prz_{c}")
                    pmm_n = psum.tile([128, 256], f32, tag=f"n_{c}", name=f"pn_{c}")

                    # rz block: xw fold (K=4 identity) + 8 ktiles
                    for j in range(NG):
                        nc.tensor.matmul(
                            pmm_rz[32 * j:32 * j + NL, :],
                            ident_sb[:, 32 * j:32 * j + NL],
                            xw_t[:, 0:512],
                            start=True, stop=False,
                            tile_position=(0, 32 * j),
                        )
                    for ki in range(KT):
                        lhsT = (hTa[c][:, 32 * ki:32 * ki + NL] if ki < 4
                                else hTb[c][:, 32 * (ki - 4):32 * (ki - 4) + NL])
                        for j in range(NG):
                            nc.tensor.matmul(
                                pmm_rz[32 * j:32 * j + NL, :],
                                lhsT,
                                whh_sb[g][:, 3 * H * ki + GW * j: 3 * H * ki + GW * j + 512],
                                start=False, stop=(ki == KT - 1),
                                tile_position=(0, 32 * j),
                            )
                    # n block: bhh_n fold (K=1 ones) + 8 ktiles
                    for j in range(NG):
                        nc.tensor.matmul(
                            pmm_n[32 * j:32 * j + NL, :],
                            ident_sb[:, 32 * j:32 * j + NL],
                            bhhn_sb[g][:, :],
                            start=True, stop=False,
                            tile_position=(0, 32 * j),
                        )
                    for ki in range(KT):
                        lhsT = (hTa[c][:, 32 * ki:32 * ki + NL] if ki < 4
                                else hTb[c][:, 32 * (ki - 4):32 * (ki - 4) + NL])
                        for j in range(NG):
                            nc.tensor.matmul(
                                pmm_n[32 * j:32 * j + NL, :],
                                lhsT,
                                whh_sb[g][:, 3 * H * ki + GW * j + 512: 3 * H * ki + GW * (j + 1)],
                                start=False, stop=(ki == KT - 1),
                                tile_position=(0, 32 * j),
                            )

                    # gate chain (strip view; only partitions 32j+b, b<4 valid)
                    rz = wpool.tile([128, 512], f16, tag=f"rz_s{c}", name=f"rz_s{c}")
                    nc.scalar.activation(rz[:], pmm_rz[:], AF.Sigmoid)
                    tn2 = wpool.tile([128, 256], f16, tag=f"tn2_{c}", name=f"tn2_{c}")
                    nc.vector.tensor_mul(tn2[:], pmm_n[:], rz[:, 0:256])
                    sn = wpool.tile([128, 256], f16, tag=f"sn_{c}", name=f"sn_{c}")
                    nc.vector.tensor_add(sn[:], tn2[:], xw_t[:, 512:768])
                    n_t = wpool.tile([128, 256], f16, tag=f"n_{c}", name=f"n_{c}")
                    nc.scalar.activation(n_t[:], sn[:], AF.Tanh)
                    d_t = wpool.tile([128, 256], f16, tag=f"d_{c}", name=f"d_{c}")
                    nc.vector.tensor_sub(d_t[:], n_t[:], h_str[c][:])
                    e_t = wpool.tile([128, 256], f16, tag=f"e_{c}", name=f"e_{c}")
                    nc.vector.tensor_mul(e_t[:], d_t[:], rz[:, 256:512])
                    h_new = wpool.tile([128, 256], f16, tag=f"h_{c}", name=f"hn_{c}")
                    nc.vector.scalar_tensor_tensor(
                        h_new[:], e_t[:], masks_sb[c][:, mcol:mcol + 1], h_str[c][:],
                        AO.mult, AO.add)
                    hTa_new = wpool.tile([128, 128], f16, tag=f"hTa_{c}", name=f"hTa_{c}")
                    nc.vector.transpose(hTa_new[:], h_new[:, 0:128])
                    hTb_new = wpool.tile([128, 128], f16, tag=f"hTb_{c}", name=f"hTb_{c}")
                    nc.vector.transpose(hTb_new[:], h_new[:, 128:256])

                    h_str[c], hTa[c], hTb[c] = h_new, hTa_new, hTb_new

            # ---- head (per chain) ---------------------------------------
            for c in range(NCH):
                ph = psum.tile([128, 512], f32, tag=f"rz_{c}", name=f"ph_{c}")
                for ki in range(KT):
                    lhsT = (hTa[c][:, 32 * ki:32 * ki + NL] if ki < 4
                            else hTb[c][:, 32 * (ki - 4):32 * (ki - 4) + NL])
                    nc.tensor.matmul(
                        ph[0:NL, 0:64],
                        lhsT,
                        p1T_sb[:, 64 * ki:64 * (ki + 1)],
                        start=(ki == 0), stop=(ki == KT - 1),
                    )
                t1s = wpool.tile([128, 64], f16, tag=f"t1s_{c}", name=f"t1s_{c}")
                nc.vector.tensor_add(t1s[0:NL, :], ph[0:NL, 0:64], p1b_sb[0:NL, :])
                t1 = wpool.tile([128, 64], f16, tag=f"t1_{c}", name=f"t1_{c}")
                nc.scalar.activation(t1[0:NL, :], t1s[0:NL, :], AF.Tanh)

                pt1 = psum.tile([128, 256], f16, tag="tp", name=f"pt1_{c}")
                nc.tensor.transpose(pt1[0:64, 0:NL], t1[0:NL, 0:64],
                                    ident_sb[0:NL, 0:NL])
                t1T = wpool.tile([64, NL], f16, tag=f"t1T_{c}", name=f"t1T_{c}")
                nc.vector.tensor_copy(t1T[:], pt1[0:64, 0:NL])

                pl = psum.tile([128, 512], f32, tag=f"rz_{c}", name=f"pl_{c}")
                nc.tensor.matmul(pl[0:NL, 0:2], t1T[:], p2T_sb[:], start=True, stop=True)
                lg = wpool.tile([128, 2], f32, tag=f"lg_{c}", name=f"lg_{c}")
                nc.vector.tensor_add(lg[0:NL, :], pl[0:NL, 0:2], p2b_sb[0:NL, :])
                nc.sync.dma_start(d_logits[NL * c:NL * (c + 1), :], lg[0:NL, :])

    nc.compile()
    return nc


# ----------------------------------------------------------------------------
# entry point
# ----------------------------------------------------------------------------

@functools.lru_cache(maxsize=2)
def _cached_program(n_steps):
    return build_program(n_steps)


def run(inputs, n_steps=T, trace=False):
    inputs = {k: np.asarray(v) for k, v in inputs.items()}
    nc = _cached_program(n_steps)
    shared = _prep_shared(inputs, n_steps)
    emb16 = np.asarray(inputs["emb"]).astype(np.float16)
    in_maps = []
    for c in range(NCORES):
        m = dict(shared)
        m.update(_prep_core(inputs, emb16, c, n_steps))
        in_maps.append(m)
    res = run_bass_kernel_spmd(nc, in_maps, list(range(NCORES)), trace=trace)
    logits = np.concatenate([res.results[c]["logits"] for c in range(NCORES)], axis=0)
    return logits, res


def kernel(**inputs) -> np.ndarray:
    logits, _ = run(inputs)
    return logits.astype(np.float32)


# revision 12
# speedup vs baseline: 1.3961x; 1.0579x over previous
"""Trainium2 Bass kernel for nn_Disc_53515292508892 (ragged_sequence).

Computes: src-GRU (H=1024) over ragged [128,64] token batch -> final hidden,
tgt-GRU seeded with it, then a 2-layer head -> logits [64, 2].
(The reference's ref-encoder outputs are dead code -- skipped.)

Sharding: data-parallel over batch, B=64 -> 8 sequences per NeuronCore,
GRU weights replicated, no inter-core communication.

v2 design (two interleaved chains):
  - The per-step recurrence is latency-bound: weight stream (~2.9us warm)
    plus a serial gate chain (~2.9us) that idles the PE and lets HAM
    re-throttle it to 1.2 GHz.  Fix: split the core's 8 lanes into TWO
    independent 4-lane chains and interleave their steps -- chain 0's gate
    chain runs on DVE/ACT while chain 1's weight stream keeps the PE busy
    (and warm).  PE ~100% occupied; per-step cost -> ~stream time.
  - xw (input-gate preacts incl. biases) folded into PSUM via a K=4
    identity matmul; bhh_n folded via a K=1 ones matmul.  Kills two DVE
    adds per step; sigmoid reads PSUM directly.
  - hT (matmul lhsT layout) produced by DVE 32x32-block transpose
    (nc.vector.transpose) straight SBUF->SBUF.  The Whh k-dim row
    permutation is chosen so block-transposed h IS a valid lhsT:
    k-tile c, partition p=32J+x  <->  hidden unit 256J+32c+x.
  - Gate column order per PE quadrant J: [r(256) | z'(256) | n(256)] for
    hidden group J (units 256J..256J+256); z' columns sign-flipped so
    z' = 1-z = sigmoid(-pre_z).

Per chain-step: PE streams rz (fold + 8 ktiles, N=512/quadrant) then n
(fold + 8 ktiles, N=256/quadrant); chain: sigmoid(psum_rz) -> tn2 =
psum_n*r -> sn = tn2+xw_n -> n = tanh(sn) -> d = n-h -> e = d*z' ->
h' = mask*e + h -> hTa/hTb = block-transpose(h').
"""

import sys
import functools

sys.path.insert(0, "/opt/trn_rl_repo")

import numpy as np
import concourse.mybir as mybir
from concourse import bacc, tile
from concourse.bass_utils import run_bass_kernel_spmd

f16 = mybir.dt.float16
f32 = mybir.dt.float32
AO = mybir.AluOpType
AF = mybir.ActivationFunctionType

V, D, H = 32000, 512, 1024
T = 128          # steps per GRU (T_SRC = T_TGT = 128)
BL = 8           # batch per core
NCH = 2          # interleaved chains per core
NL = BL // NCH   # lanes per chain (4)
NCORES = 8
NG = 4           # col-tile quadrants
GW = 768         # gate columns per quadrant (256 r | 256 z' | 256 n)
KT = H // 128    # 8 k-tiles over hidden
KD = D // 128    # 4 k-tiles over embedding dim


# ----------------------------------------------------------------------------
# host-side weight/layout prep
# ----------------------------------------------------------------------------

def _col_perm():
    """newcol g in [0,3072): quadrant j=g//768, gate=(g%768)//256, u=g%256.
    orig W row = gate_base + 256j + u;  sign=-1 for z columns; is_n mask."""
    g = np.arange(3 * H)
    j = g // GW
    rem = g % GW
    gate = rem // 256
    u = rem % 256
    row = gate * H + 256 * j + u
    sign = np.where(gate == 1, -1.0, 1.0).astype(np.float32)
    is_n = gate == 2
    return row, sign, is_n


def _k_unit(c):
    """hidden unit held by partition p for k-tile c: 256*(p//32)+32c+(p%32)."""
    p = np.arange(128)
    return 256 * (p // 32) + 32 * c + (p % 32)


def _prep_shared(inputs, n_steps):
    row, sign, is_n = _col_perm()
    out = {}
    for g, wih, whh, bih, bhh in (
        ("src", inputs["src_Wih"], inputs["src_Whh"], inputs["src_bih"], inputs["src_bhh"]),
        ("tgt", inputs["tgt_Wih"], inputs["tgt_Whh"], inputs["tgt_bih"], inputs["tgt_bhh"]),
    ):
        wcol = whh[row] * sign[:, None]                       # [3072, 1024]
        whh_a = np.empty((KT, 128, 3 * H), np.float16)
        for c in range(KT):
            whh_a[c] = wcol[:, _k_unit(c)].T.astype(np.float16)
        out[f"whh_{g}"] = np.ascontiguousarray(whh_a)
        wih_a = (wih[row] * sign[:, None]).T.astype(np.float16)   # [512, 3072]
        out[f"wih_{g}"] = np.ascontiguousarray(wih_a.reshape(KD, 128, 3 * H))
        bias_vec = sign * bih[row] + sign * np.where(is_n, 0.0, bhh[row])
        out[f"bias_{g}"] = np.broadcast_to(
            bias_vec.astype(np.float16), (128, 3 * H)).copy()
        # bhh_n broadcast strip for the K=128 identity fold: rows 32j+b
        bhhn = np.zeros((128, 256), np.float16)
        for j in range(NG):
            bhhn[32 * j:32 * j + 32, :] = bhh[2 * H + 256 * j: 2 * H + 256 * (j + 1)].astype(np.float16)
        out[f"bhhn_{g}"] = bhhn
    p1 = np.empty((128, KT * 64), np.float32)
    for c in range(KT):
        p1[:, 64 * c:64 * (c + 1)] = inputs["p1_W"][:, _k_unit(c)].T
    out["p1T"] = p1.astype(np.float16)
    out["p1b"] = np.broadcast_to(inputs["p1_b"].astype(np.float16), (128, 64)).copy()
    out["p2T"] = inputs["p2_W"].T.astype(np.float16)                  # [64, 2]
    out["p2b"] = np.broadcast_to(inputs["p2_b"].astype(np.float32), (128, 2)).copy()
    out["ident"] = np.eye(128, dtype=np.float16)
    return out


def _prep_core(inputs, emb16, core, n_steps):
    """Per-core tensors: per-chain gathered/transposed embeddings and masks."""
    out = {}
    for ch in range(NCH):
        sl = slice(BL * core + NL * ch, BL * core + NL * (ch + 1))
        for g, ids_key in (("src", "src"), ("tgt", "tgt")):
            ids = np.asarray(inputs[ids_key])[:n_steps, sl]           # [T, 4]
            x = emb16[ids]                                            # [T, 4, 512]
            out[f"xT_{g}_{ch}"] = np.ascontiguousarray(
                x.transpose(2, 0, 1).reshape(KD, 128, n_steps * NL))
        masks = np.zeros((128, 2 * n_steps), np.float32)
        for gi, len_key in enumerate(("src_lengths", "tgt_lengths")):
            ln = np.asarray(inputs[len_key])[sl]                      # [4]
            t = np.arange(n_steps)
            m = (t[None, :] < ln[:, None]).astype(np.float32)         # [4, T]
            for j in range(NG):
                masks[32 * j:32 * j + NL, gi * n_steps:(gi + 1) * n_steps] = m
        out[f"masks_{ch}"] = masks
    return out


# ----------------------------------------------------------------------------
# device program
# ----------------------------------------------------------------------------

def build_program(n_steps=T):
    nc = bacc.Bacc("TRN2", target_bir_lowering=False, debug=False,
                   num_devices=NCORES)
    TBc = n_steps * NL   # 512 rows of xw per chain per GRU

    dp = nc.declare_dram_parameter
    d_xT = {(g, c): dp(f"xT_{g}_{c}", [KD, 128, TBc], f16, isOutput=False)
            for g in ("src", "tgt") for c in range(NCH)}
    d_whh = {g: dp(f"whh_{g}", [KT, 128, 3 * H], f16, isOutput=False) for g in ("src", "tgt")}
    d_wih = {g: dp(f"wih_{g}", [KD, 128, 3 * H], f16, isOutput=False) for g in ("src", "tgt")}
    d_bias = {g: dp(f"bias_{g}", [128, 3 * H], f16, isOutput=False) for g in ("src", "tgt")}
    d_bhhn = {g: dp(f"bhhn_{g}", [128, 256], f16, isOutput=False) for g in ("src", "tgt")}
    d_masks = {c: dp(f"masks_{c}", [128, 2 * n_steps], f32, isOutput=False)
               for c in range(NCH)}
    d_p1T = dp("p1T", [128, KT * 64], f16, isOutput=False)
    d_p1b = dp("p1b", [128, 64], f16, isOutput=False)
    d_p2T = dp("p2T", [64, 2], f16, isOutput=False)
    d_p2b = dp("p2b", [128, 2], f32, isOutput=False)
    d_ident = dp("ident", [128, 128], f16, isOutput=False)
    d_logits = dp("logits", [BL, 2], f32, isOutput=True)

    with tile.TileContext(nc) as tc:
        with tc.tile_pool(name="const", bufs=1) as cpool, \
             tc.tile_pool(name="work", bufs=2) as wpool, \
             tc.tile_pool(name="xwload", bufs=4) as xwpool, \
             tc.tile_pool(name="p1ev", bufs=4) as evpool, \
             tc.tile_pool(name="psum", bufs=1, space="PSUM") as psum, \
             tc.tile_pool(name="dram", bufs=1, space="DRAM") as dram:

            # ---- resident constants -------------------------------------
            # phase-1 inputs (xT/bias) load FIRST; the 12.6MB whh preload
            # goes last -- it is only needed when the recurrence starts
            whh_sb, xT_sb, bias_sb, bhhn_sb = {}, {}, {}, {}
            for g in ("src", "tgt"):
                for c in range(NCH):
                    xT_sb[(g, c)] = cpool.tile([128, KD * TBc], f16,
                                               tag=f"xT_{g}_{c}", name=f"xT_{g}_{c}")
                    for k in range(KD):
                        nc.sync.dma_start(
                            xT_sb[(g, c)][:, TBc * k:TBc * (k + 1)], d_xT[(g, c)][k])
                bias_sb[g] = cpool.tile([128, 3 * H], f16, tag=f"bias_{g}", name=f"biassb_{g}")
                nc.sync.dma_start(bias_sb[g][:], d_bias[g][:])
                bhhn_sb[g] = cpool.tile([128, 256], f16, tag=f"bhhn_{g}", name=f"bhhnsb_{g}")
                nc.sync.dma_start(bhhn_sb[g][:], d_bhhn[g][:])
            for g in ("src", "tgt"):
                whh_sb[g] = cpool.tile([128, KT * 3 * H], f16, tag=f"whh_{g}", name=f"whh_{g}")
            masks_sb = {}
            for c in range(NCH):
                masks_sb[c] = cpool.tile([128, 2 * n_steps], f32, tag=f"masks_{c}",
                                         name=f"masks_{c}")
                nc.sync.dma_start(masks_sb[c][:], d_masks[c][:])
            ident_sb = cpool.tile([128, 128], f16, tag="ident")
            nc.sync.dma_start(ident_sb[:], d_ident[:])
            p1T_sb = cpool.tile([128, KT * 64], f16, tag="p1T")
            nc.sync.dma_start(p1T_sb[:], d_p1T[:])
            p1b_sb = cpool.tile([128, 64], f16, tag="p1b")
            nc.sync.dma_start(p1b_sb[:], d_p1b[:])
            p2T_sb = cpool.tile([64, 2], f16, tag="p2T")
            nc.sync.dma_start(p2T_sb[:], d_p2T[:])
            p2b_sb = cpool.tile([128, 2], f32, tag="p2b")
            nc.sync.dma_start(p2b_sb[:], d_p2b[:])
            for g in ("src", "tgt"):
                for k in range(KT):
                    nc.gpsimd.dma_start(whh_sb[g][:, 3 * H * k:3 * H * (k + 1)], d_whh[g][k])

            # ---- phase 1: xw = x @ Wih.T + bias -> DRAM -----------------
            xw_dram = {}
            for g in ("src", "tgt"):
                for c in range(NCH):
                    xw_dram[(g, c)] = dram.tile([TBc, 3 * H], f16,
                                                tag=f"xw_{g}_{c}", name=f"xwdram_{g}_{c}")
            n_mstrip = TBc // 128  # 4

            for g in ("src", "tgt"):
                for chk in range(6):
                    wihs = []
                    for kd in range(KD):
                        wt = evpool.tile([128, 512], f16, tag="wih_s", name="wih_s",
                                         bufs=8)
                        nc.sync.dma_start(
                            wt[:], d_wih[g][kd, :, 512 * chk:512 * (chk + 1)])
                        wihs.append(wt)
                    for c in range(NCH):
                        for mi in range(n_mstrip):
                            m0 = 128 * mi
                            ps = psum.tile([128, 512], f32, tag="p1", name="p1ps", bufs=2)
                            for kd in range(KD):
                                nc.tensor.matmul(
                                    ps[:],
                                    xT_sb[(g, c)][:, TBc * kd + m0: TBc * kd + m0 + 128],
                                    wihs[kd][:],
                                    start=(kd == 0), stop=(kd == KD - 1),
                                )
                            ev = evpool.tile([128, 512], f16, tag="ev", name="ev")
                            nc.vector.tensor_add(
                                ev[:], ps[:],
                                bias_sb[g][:, 512 * chk:512 * (chk + 1)])
                            nc.sync.dma_start(
                                xw_dram[(g, c)][m0:m0 + 128, 512 * chk:512 * (chk + 1)],
                                ev[:])

            # ---- recurrence: two interleaved 4-lane chains --------------
            h_str, hTa, hTb = {}, {}, {}
            for c in range(NCH):
                h_str[c] = wpool.tile([128, 256], f16, tag=f"h_{c}", name=f"h0_{c}")
                hTa[c] = wpool.tile([128, 128], f16, tag=f"hTa_{c}", name=f"hTa0_{c}")
                hTb[c] = wpool.tile([128, 128], f16, tag=f"hTb_{c}", name=f"hTb0_{c}")
                nc.vector.memset(h_str[c][:], 0.0)
                nc.vector.memset(hTa[c][:], 0.0)
                nc.vector.memset(hTb[c][:], 0.0)

            for c in range(NCH):
                for _ in range(4):
                    warm = xwpool.tile([128, GW], f16, tag=f"xw_{c}", name=f"xww_{c}")
                    nc.vector.memset(warm[:], 0.0)

            for step in range(2 * n_steps):
                g = "src" if step < n_steps else "tgt"
                t = step % n_steps
                mcol = t if g == "src" else n_steps + t

                for c in range(NCH):
                    xw_t = xwpool.tile([128, GW], f16, tag=f"xw_{c}", name=f"xw_{c}")
                    for j in range(NG):
                        nc.sync.dma_start(
                            xw_t[32 * j:32 * j + NL, :],
                            xw_dram[(g, c)][t * NL:(t + 1) * NL, GW * j:GW * (j + 1)])

                    pmm_rz = psum.tile([128, 512], f32, tag=f"rz_{c}", name=f"prz_{c}")
                    pmm_n = psum.tile([128, 256], f32, tag=f"n_{c}", name=f"pn_{c}")

                    # rz block: xw fold (K=4 identity) + 8 ktiles
                    for j in range(NG):
                        nc.tensor.matmul(
                            pmm_rz[32 * j:32 * j + NL, :],
                            ident_sb[:, 32 * j:32 * j + NL],
                            xw_t[:, 0:512],
                            start=True, stop=False,
                            tile_position=(0, 32 * j),
                        )
                    for ki in range(KT):
                        lhsT = (hTa[c][:, 32 * ki:32 * ki + NL] if ki < 4
                                else hTb[c][:, 32 * (ki - 4):32 * (ki - 4) + NL])
                        for j in range(NG):
                            nc.tensor.matmul(
                                pmm_rz[32 * j:32 * j + NL, :],
                                lhsT,
                                whh_sb[g][:, 3 * H * ki + GW * j: 3 * H * ki + GW * j + 512],
                                start=False, stop=(ki == KT - 1),
                                tile_position=(0, 32 * j),
                            )
                    # n block: bhh_n fold (K=1 ones) + 8 ktiles
                    for j in range(NG):
                        nc.tensor.matmul(
                            pmm_n[32 * j:32 * j + NL, :],
                            ident_sb[:, 32 * j:32 * j + NL],
                            bhhn_sb[g][:, :],
                            start=True, stop=False,
                            tile_position=(0, 32 * j),
                        )
                    for ki in range(KT):
                        lhsT = (hTa[c][:, 32 * ki:32 * ki + NL] if ki < 4
                                else hTb[c][:, 32 * (ki - 4):32 * (ki - 4) + NL])
                        for j in range(NG):
                            nc.tensor.matmul(
                                pmm_n[32 * j:32 * j + NL, :],
                                lhsT,
                                whh_sb[g][:, 3 * H * ki + GW * j + 512: 3 * H * ki + GW * (j + 1)],
                                start=False, stop=(ki == KT - 1),
                                tile_position=(0, 32 * j),
                            )

                    # gate chain (strip view; only partitions 32j+b, b<4 valid)
                    rz = wpool.tile([128, 512], f16, tag=f"rz_s{c}", name=f"rz_s{c}")
                    nc.scalar.activation(rz[:, 0:256], pmm_rz[:, 0:256], AF.Sigmoid)
                    nc.scalar.activation(rz[:, 256:512], pmm_rz[:, 256:512], AF.Sigmoid)
                    tn2 = wpool.tile([128, 256], f16, tag=f"tn2_{c}", name=f"tn2_{c}")
                    nc.vector.tensor_mul(tn2[:], pmm_n[:], rz[:, 0:256])
                    sn = wpool.tile([128, 256], f16, tag=f"sn_{c}", name=f"sn_{c}")
                    nc.vector.tensor_add(sn[:], tn2[:], xw_t[:, 512:768])
                    # zm = z' * mask (off critical path); tail split into
                    # halves so vtr_a (gating next step's ktile-0) lands early
                    n_t = wpool.tile([128, 256], f16, tag=f"n_{c}", name=f"n_{c}")
                    zm = wpool.tile([128, 256], f16, tag=f"zm_{c}", name=f"zm_{c}")
                    h_new = wpool.tile([128, 256], f16, tag=f"h_{c}", name=f"hn_{c}")
                    hTa_new = wpool.tile([128, 128], f16, tag=f"hTa_{c}", name=f"hTa_{c}")
                    hTb_new = wpool.tile([128, 128], f16, tag=f"hTb_{c}", name=f"hTb_{c}")
                    nc.scalar.activation(n_t[:, 0:128], sn[:, 0:128], AF.Tanh)
                    nc.vector.tensor_scalar_mul(
                        zm[:], rz[:, 256:512], masks_sb[c][:, mcol:mcol + 1])
                    for o, hT_new in ((0, hTa_new), (128, hTb_new)):
                        if o:
                            nc.scalar.activation(n_t[:, o:o + 128], sn[:, o:o + 128],
                                                 AF.Tanh)
                        d_t = wpool.tile([128, 128], f16, tag=f"d_{c}{o}", name=f"d_{c}{o}")
                        nc.vector.tensor_sub(d_t[:], n_t[:, o:o + 128],
                                             h_str[c][:, o:o + 128])
                        e_t = wpool.tile([128, 128], f16, tag=f"e_{c}{o}", name=f"e_{c}{o}")
                        nc.vector.tensor_mul(e_t[:], d_t[:], zm[:, o:o + 128])
                        nc.vector.tensor_add(h_new[:, o:o + 128], e_t[:],
                                             h_str[c][:, o:o + 128])
                        nc.vector.transpose(hT_new[:], h_new[:, o:o + 128])

                    h_str[c], hTa[c], hTb[c] = h_new, hTa_new, hTb_new

            # ---- head (per chain) ---------------------------------------
            for c in range(NCH):
                ph = psum.tile([128, 512], f32, tag=f"rz_{c}", name=f"ph_{c}")
                for ki in range(KT):
                    lhsT = (hTa[c][:, 32 * ki:32 * ki + NL] if ki < 4
                            else hTb[c][:, 32 * (ki - 4):32 * (ki - 4) + NL])
                    nc.tensor.matmul(
                        ph[0:NL, 0:64],
                        lhsT,
                        p1T_sb[:, 64 * ki:64 * (ki + 1)],
                        start=(ki == 0), stop=(ki == KT - 1),
                    )
                t1s = wpool.tile([128, 64], f16, tag=f"t1s_{c}", name=f"t1s_{c}")
                nc.vector.tensor_add(t1s[0:NL, :], ph[0:NL, 0:64], p1b_sb[0:NL, :])
                t1 = wpool.tile([128, 64], f16, tag=f"t1_{c}", name=f"t1_{c}")
                nc.scalar.activation(t1[0:NL, :], t1s[0:NL, :], AF.Tanh)

                pt1 = psum.tile([128, 256], f16, tag="tp", name=f"pt1_{c}")
                nc.tensor.transpose(pt1[0:64, 0:NL], t1[0:NL, 0:64],
                                    ident_sb[0:NL, 0:NL])
                t1T = wpool.tile([64, NL], f16, tag=f"t1T_{c}", name=f"t1T_{c}")
                nc.vector.tensor_copy(t1T[:], pt1[0:64, 0:NL])

                pl = psum.tile([128, 512], f32, tag=f"rz_{c}", name=f"pl_{c}")
                nc.tensor.matmul(pl[0:NL, 0:2], t1T[:], p2T_sb[:], start=True, stop=True)
                lg = wpool.tile([128, 2], f32, tag=f"lg_{c}", name=f"lg_{c}")
                nc.vector.tensor_add(lg[0:NL, :], pl[0:NL, 0:2], p2b_sb[0:NL, :])
                nc.sync.dma_start(d_logits[NL * c:NL * (c + 1), :], lg[0:NL, :])

    nc.compile()
    return nc


# ----------------------------------------------------------------------------
# entry point
# ----------------------------------------------------------------------------

@functools.lru_cache(maxsize=2)
def _cached_program(n_steps):
    return build_program(n_steps)


def run(inputs, n_steps=T, trace=False):
    inputs = {k: np.asarray(v) for k, v in inputs.items()}
    nc = _cached_program(n_steps)
    shared = _prep_shared(inputs, n_steps)
    emb16 = np.asarray(inputs["emb"]).astype(np.float16)
    in_maps = []
    for c in range(NCORES):
        m = dict(shared)
        m.update(_prep_core(inputs, emb16, c, n_steps))
        in_maps.append(m)
    res = run_bass_kernel_spmd(nc, in_maps, list(range(NCORES)), trace=trace)
    logits = np.concatenate([res.results[c]["logits"] for c in range(NCORES)], axis=0)
    return logits, res


def kernel(**inputs) -> np.ndarray:
    logits, _ = run(inputs)
    return logits.astype(np.float32)


# revision 13
# speedup vs baseline: 1.3981x; 1.0014x over previous
"""Trainium2 Bass kernel for nn_Disc_53515292508892 (ragged_sequence).

Computes: src-GRU (H=1024) over ragged [128,64] token batch -> final hidden,
tgt-GRU seeded with it, then a 2-layer head -> logits [64, 2].
(The reference's ref-encoder outputs are dead code -- skipped.)

Sharding: data-parallel over batch, B=64 -> 8 sequences per NeuronCore,
GRU weights replicated, no inter-core communication.

v2 design (two interleaved chains):
  - The per-step recurrence is latency-bound: weight stream (~2.9us warm)
    plus a serial gate chain (~2.9us) that idles the PE and lets HAM
    re-throttle it to 1.2 GHz.  Fix: split the core's 8 lanes into TWO
    independent 4-lane chains and interleave their steps -- chain 0's gate
    chain runs on DVE/ACT while chain 1's weight stream keeps the PE busy
    (and warm).  PE ~100% occupied; per-step cost -> ~stream time.
  - xw (input-gate preacts incl. biases) folded into PSUM via a K=4
    identity matmul; bhh_n folded via a K=1 ones matmul.  Kills two DVE
    adds per step; sigmoid reads PSUM directly.
  - hT (matmul lhsT layout) produced by DVE 32x32-block transpose
    (nc.vector.transpose) straight SBUF->SBUF.  The Whh k-dim row
    permutation is chosen so block-transposed h IS a valid lhsT:
    k-tile c, partition p=32J+x  <->  hidden unit 256J+32c+x.
  - Gate column order per PE quadrant J: [r(256) | z'(256) | n(256)] for
    hidden group J (units 256J..256J+256); z' columns sign-flipped so
    z' = 1-z = sigmoid(-pre_z).

Per chain-step: PE streams rz (fold + 8 ktiles, N=512/quadrant) then n
(fold + 8 ktiles, N=256/quadrant); chain: sigmoid(psum_rz) -> tn2 =
psum_n*r -> sn = tn2+xw_n -> n = tanh(sn) -> d = n-h -> e = d*z' ->
h' = mask*e + h -> hTa/hTb = block-transpose(h').
"""

import sys
import functools

sys.path.insert(0, "/opt/trn_rl_repo")

import numpy as np
import concourse.mybir as mybir
from concourse import bacc, tile
from concourse.bass_utils import run_bass_kernel_spmd

f16 = mybir.dt.float16
f32 = mybir.dt.float32
AO = mybir.AluOpType
AF = mybir.ActivationFunctionType

V, D, H = 32000, 512, 1024
T = 128          # steps per GRU (T_SRC = T_TGT = 128)
BL = 8           # batch per core
NCH = 2          # interleaved chains per core
NL = BL // NCH   # lanes per chain (4)
NCORES = 8
NG = 4           # col-tile quadrants
GW = 768         # gate columns per quadrant (256 r | 256 z' | 256 n)
KT = H // 128    # 8 k-tiles over hidden
KD = D // 128    # 4 k-tiles over embedding dim


# ----------------------------------------------------------------------------
# host-side weight/layout prep
# ----------------------------------------------------------------------------

def _col_perm():
    """newcol g in [0,3072): quadrant j=g//768, gate=(g%768)//256, u=g%256.
    orig W row = gate_base + 256j + u;  sign=-1 for z columns; is_n mask."""
    g = np.arange(3 * H)
    j = g // GW
    rem = g % GW
    gate = rem // 256
    u = rem % 256
    row = gate * H + 256 * j + u
    sign = np.where(gate == 1, -1.0, 1.0).astype(np.float32)
    is_n = gate == 2
    return row, sign, is_n


def _k_unit(c):
    """hidden unit held by partition p for k-tile c: 256*(p//32)+32c+(p%32)."""
    p = np.arange(128)
    return 256 * (p // 32) + 32 * c + (p % 32)


def _prep_shared(inputs, n_steps):
    row, sign, is_n = _col_perm()
    out = {}
    for g, wih, whh, bih, bhh in (
        ("src", inputs["src_Wih"], inputs["src_Whh"], inputs["src_bih"], inputs["src_bhh"]),
        ("tgt", inputs["tgt_Wih"], inputs["tgt_Whh"], inputs["tgt_bih"], inputs["tgt_bhh"]),
    ):
        wcol = whh[row] * sign[:, None]                       # [3072, 1024]
        whh_a = np.empty((KT, 128, 3 * H), np.float16)
        for c in range(KT):
            whh_a[c] = wcol[:, _k_unit(c)].T.astype(np.float16)
        out[f"whh_{g}"] = np.ascontiguousarray(whh_a)
        wih_a = (wih[row] * sign[:, None]).T.astype(np.float16)   # [512, 3072]
        out[f"wih_{g}"] = np.ascontiguousarray(wih_a.reshape(KD, 128, 3 * H))
        bias_vec = sign * bih[row] + sign * np.where(is_n, 0.0, bhh[row])
        out[f"bias_{g}"] = np.broadcast_to(
            bias_vec.astype(np.float16), (128, 3 * H)).copy()
        # bhh_n broadcast strip for the K=128 identity fold: rows 32j+b
        bhhn = np.zeros((128, 256), np.float16)
        for j in range(NG):
            bhhn[32 * j:32 * j + 32, :] = bhh[2 * H + 256 * j: 2 * H + 256 * (j + 1)].astype(np.float16)
        out[f"bhhn_{g}"] = bhhn
    p1 = np.empty((128, KT * 64), np.float32)
    for c in range(KT):
        p1[:, 64 * c:64 * (c + 1)] = inputs["p1_W"][:, _k_unit(c)].T
    out["p1T"] = p1.astype(np.float16)
    out["p1b"] = np.broadcast_to(inputs["p1_b"].astype(np.float16), (128, 64)).copy()
    out["p2T"] = inputs["p2_W"].T.astype(np.float16)                  # [64, 2]
    out["p2b"] = np.broadcast_to(inputs["p2_b"].astype(np.float32), (128, 2)).copy()
    out["ident"] = np.eye(128, dtype=np.float16)
    return out


def _prep_core(inputs, emb16, core, n_steps):
    """Per-core tensors: per-chain gathered/transposed embeddings and masks."""
    out = {}
    for ch in range(NCH):
        sl = slice(BL * core + NL * ch, BL * core + NL * (ch + 1))
        for g, ids_key in (("src", "src"), ("tgt", "tgt")):
            ids = np.asarray(inputs[ids_key])[:n_steps, sl]           # [T, 4]
            x = emb16[ids]                                            # [T, 4, 512]
            out[f"xT_{g}_{ch}"] = np.ascontiguousarray(
                x.transpose(2, 0, 1).reshape(KD, 128, n_steps * NL))
        masks = np.zeros((128, 2 * n_steps), np.float32)
        for gi, len_key in enumerate(("src_lengths", "tgt_lengths")):
            ln = np.asarray(inputs[len_key])[sl]                      # [4]
            t = np.arange(n_steps)
            m = (t[None, :] < ln[:, None]).astype(np.float32)         # [4, T]
            for j in range(NG):
                masks[32 * j:32 * j + NL, gi * n_steps:(gi + 1) * n_steps] = m
        out[f"masks_{ch}"] = masks
    return out


# ----------------------------------------------------------------------------
# device program
# ----------------------------------------------------------------------------

def build_program(n_steps=T):
    nc = bacc.Bacc("TRN2", target_bir_lowering=False, debug=False,
                   num_devices=NCORES)
    TBc = n_steps * NL   # 512 rows of xw per chain per GRU

    dp = nc.declare_dram_parameter
    d_xT = {(g, c): dp(f"xT_{g}_{c}", [KD, 128, TBc], f16, isOutput=False)
            for g in ("src", "tgt") for c in range(NCH)}
    d_whh = {g: dp(f"whh_{g}", [KT, 128, 3 * H], f16, isOutput=False) for g in ("src", "tgt")}
    d_wih = {g: dp(f"wih_{g}", [KD, 128, 3 * H], f16, isOutput=False) for g in ("src", "tgt")}
    d_bias = {g: dp(f"bias_{g}", [128, 3 * H], f16, isOutput=False) for g in ("src", "tgt")}
    d_bhhn = {g: dp(f"bhhn_{g}", [128, 256], f16, isOutput=False) for g in ("src", "tgt")}
    d_masks = {c: dp(f"masks_{c}", [128, 2 * n_steps], f32, isOutput=False)
               for c in range(NCH)}
    d_p1T = dp("p1T", [128, KT * 64], f16, isOutput=False)
    d_p1b = dp("p1b", [128, 64], f16, isOutput=False)
    d_p2T = dp("p2T", [64, 2], f16, isOutput=False)
    d_p2b = dp("p2b", [128, 2], f32, isOutput=False)
    d_ident = dp("ident", [128, 128], f16, isOutput=False)
    d_logits = dp("logits", [BL, 2], f32, isOutput=True)

    with tile.TileContext(nc) as tc:
        with tc.tile_pool(name="const", bufs=1) as cpool, \
             tc.tile_pool(name="work", bufs=2) as wpool, \
             tc.tile_pool(name="xwload", bufs=4) as xwpool, \
             tc.tile_pool(name="p1ev", bufs=4) as evpool, \
             tc.tile_pool(name="psum", bufs=1, space="PSUM") as psum, \
             tc.tile_pool(name="dram", bufs=1, space="DRAM") as dram:

            # ---- resident constants -------------------------------------
            # phase-1 inputs (xT/bias) load FIRST; the 12.6MB whh preload
            # goes last -- it is only needed when the recurrence starts
            whh_sb, xT_sb, bias_sb, bhhn_sb = {}, {}, {}, {}
            for g in ("src", "tgt"):
                for c in range(NCH):
                    xT_sb[(g, c)] = cpool.tile([128, KD * TBc], f16,
                                               tag=f"xT_{g}_{c}", name=f"xT_{g}_{c}")
                    for k in range(KD):
                        nc.sync.dma_start(
                            xT_sb[(g, c)][:, TBc * k:TBc * (k + 1)], d_xT[(g, c)][k])
                bias_sb[g] = cpool.tile([128, 3 * H], f16, tag=f"bias_{g}", name=f"biassb_{g}")
                nc.sync.dma_start(bias_sb[g][:], d_bias[g][:])
                bhhn_sb[g] = cpool.tile([128, 256], f16, tag=f"bhhn_{g}", name=f"bhhnsb_{g}")
                nc.sync.dma_start(bhhn_sb[g][:], d_bhhn[g][:])
            for g in ("src", "tgt"):
                whh_sb[g] = cpool.tile([128, KT * 3 * H], f16, tag=f"whh_{g}", name=f"whh_{g}")
            masks_sb = {}
            for c in range(NCH):
                masks_sb[c] = cpool.tile([128, 2 * n_steps], f32, tag=f"masks_{c}",
                                         name=f"masks_{c}")
                nc.sync.dma_start(masks_sb[c][:], d_masks[c][:])
            ident_sb = cpool.tile([128, 128], f16, tag="ident")
            nc.sync.dma_start(ident_sb[:], d_ident[:])
            p1T_sb = cpool.tile([128, KT * 64], f16, tag="p1T")
            nc.sync.dma_start(p1T_sb[:], d_p1T[:])
            p1b_sb = cpool.tile([128, 64], f16, tag="p1b")
            nc.sync.dma_start(p1b_sb[:], d_p1b[:])
            p2T_sb = cpool.tile([64, 2], f16, tag="p2T")
            nc.sync.dma_start(p2T_sb[:], d_p2T[:])
            p2b_sb = cpool.tile([128, 2], f32, tag="p2b")
            nc.sync.dma_start(p2b_sb[:], d_p2b[:])

            # ---- phase 1: xw = x @ Wih.T + bias -> DRAM -----------------
            xw_dram = {}
            for g in ("src", "tgt"):
                for c in range(NCH):
                    xw_dram[(g, c)] = dram.tile([TBc, 3 * H], f16,
                                                tag=f"xw_{g}_{c}", name=f"xwdram_{g}_{c}")
            n_mstrip = TBc // 128  # 4

            whh_pieces = [(g, k) for g in ("src", "tgt") for k in range(KT)]

            for g in ("src", "tgt"):
                for chk in range(6):
                    for _ in range(2):
                        if whh_pieces:
                            wg, wk = whh_pieces.pop(0)
                            nc.sync.dma_start(
                                whh_sb[wg][:, 3 * H * wk:3 * H * (wk + 1)],
                                d_whh[wg][wk])
                    wihs = []
                    for kd in range(KD):
                        wt = evpool.tile([128, 512], f16, tag="wih_s", name="wih_s",
                                         bufs=8)
                        nc.sync.dma_start(
                            wt[:], d_wih[g][kd, :, 512 * chk:512 * (chk + 1)])
                        wihs.append(wt)
                    for c in range(NCH):
                        for mi in range(n_mstrip):
                            m0 = 128 * mi
                            ps = psum.tile([128, 512], f32, tag="p1", name="p1ps", bufs=2)
                            for kd in range(KD):
                                nc.tensor.matmul(
                                    ps[:],
                                    xT_sb[(g, c)][:, TBc * kd + m0: TBc * kd + m0 + 128],
                                    wihs[kd][:],
                                    start=(kd == 0), stop=(kd == KD - 1),
                                )
                            ev = evpool.tile([128, 512], f16, tag="ev", name="ev")
                            nc.vector.tensor_add(
                                ev[:], ps[:],
                                bias_sb[g][:, 512 * chk:512 * (chk + 1)])
                            nc.sync.dma_start(
                                xw_dram[(g, c)][m0:m0 + 128, 512 * chk:512 * (chk + 1)],
                                ev[:])

            # ---- recurrence: two interleaved 4-lane chains --------------
            h_str, hTa, hTb = {}, {}, {}
            for c in range(NCH):
                h_str[c] = wpool.tile([128, 256], f16, tag=f"h_{c}", name=f"h0_{c}")
                hTa[c] = wpool.tile([128, 128], f16, tag=f"hTa_{c}", name=f"hTa0_{c}")
                hTb[c] = wpool.tile([128, 128], f16, tag=f"hTb_{c}", name=f"hTb0_{c}")
                nc.vector.memset(h_str[c][:], 0.0)
                nc.vector.memset(hTa[c][:], 0.0)
                nc.vector.memset(hTb[c][:], 0.0)

            for c in range(NCH):
                for _ in range(4):
                    warm = xwpool.tile([128, GW], f16, tag=f"xw_{c}", name=f"xww_{c}")
                    nc.vector.memset(warm[:], 0.0)

            for step in range(2 * n_steps):
                g = "src" if step < n_steps else "tgt"
                t = step % n_steps
                mcol = t if g == "src" else n_steps + t

                for c in range(NCH):
                    xw_t = xwpool.tile([128, GW], f16, tag=f"xw_{c}", name=f"xw_{c}")
                    for j in range(NG):
                        nc.sync.dma_start(
                            xw_t[32 * j:32 * j + NL, :],
                            xw_dram[(g, c)][t * NL:(t + 1) * NL, GW * j:GW * (j + 1)])

                    pmm_rz = psum.tile([128, 512], f32, tag=f"rz_{c}", name=f"prz_{c}")
                    pmm_n = psum.tile([128, 256], f32, tag=f"n_{c}", name=f"pn_{c}")

                    # rz block: xw fold (K=4 identity) + 8 ktiles
                    for j in range(NG):
                        nc.tensor.matmul(
                            pmm_rz[32 * j:32 * j + NL, :],
                            ident_sb[:, 32 * j:32 * j + NL],
                            xw_t[:, 0:512],
                            start=True, stop=False,
                            tile_position=(0, 32 * j),
                        )
                    for ki in range(KT):
                        lhsT = (hTa[c][:, 32 * ki:32 * ki + NL] if ki < 4
                                else hTb[c][:, 32 * (ki - 4):32 * (ki - 4) + NL])
                        for j in range(NG):
                            nc.tensor.matmul(
                                pmm_rz[32 * j:32 * j + NL, :],
                                lhsT,
                                whh_sb[g][:, 3 * H * ki + GW * j: 3 * H * ki + GW * j + 512],
                                start=False, stop=(ki == KT - 1),
                                tile_position=(0, 32 * j),
                            )
                    # n block: bhh_n fold (K=1 ones) + 8 ktiles
                    for j in range(NG):
                        nc.tensor.matmul(
                            pmm_n[32 * j:32 * j + NL, :],
                            ident_sb[:, 32 * j:32 * j + NL],
                            bhhn_sb[g][:, :],
                            start=True, stop=False,
                            tile_position=(0, 32 * j),
                        )
                    for ki in range(KT):
                        lhsT = (hTa[c][:, 32 * ki:32 * ki + NL] if ki < 4
                                else hTb[c][:, 32 * (ki - 4):32 * (ki - 4) + NL])
                        for j in range(NG):
                            nc.tensor.matmul(
                                pmm_n[32 * j:32 * j + NL, :],
                                lhsT,
                                whh_sb[g][:, 3 * H * ki + GW * j + 512: 3 * H * ki + GW * (j + 1)],
                                start=False, stop=(ki == KT - 1),
                                tile_position=(0, 32 * j),
                            )

                    # gate chain (strip view; only partitions 32j+b, b<4 valid)
                    rz = wpool.tile([128, 512], f16, tag=f"rz_s{c}", name=f"rz_s{c}")
                    nc.scalar.activation(rz[:, 0:256], pmm_rz[:, 0:256], AF.Sigmoid)
                    nc.scalar.activation(rz[:, 256:512], pmm_rz[:, 256:512], AF.Sigmoid)
                    tn2 = wpool.tile([128, 256], f16, tag=f"tn2_{c}", name=f"tn2_{c}")
                    nc.vector.tensor_mul(tn2[:], pmm_n[:], rz[:, 0:256])
                    sn = wpool.tile([128, 256], f16, tag=f"sn_{c}", name=f"sn_{c}")
                    nc.vector.tensor_add(sn[:], tn2[:], xw_t[:, 512:768])
                    # zm = z' * mask (off critical path); tail split into
                    # halves so vtr_a (gating next step's ktile-0) lands early
                    n_t = wpool.tile([128, 256], f16, tag=f"n_{c}", name=f"n_{c}")
                    zm = wpool.tile([128, 256], f16, tag=f"zm_{c}", name=f"zm_{c}")
                    h_new = wpool.tile([128, 256], f16, tag=f"h_{c}", name=f"hn_{c}")
                    hTa_new = wpool.tile([128, 128], f16, tag=f"hTa_{c}", name=f"hTa_{c}")
                    hTb_new = wpool.tile([128, 128], f16, tag=f"hTb_{c}", name=f"hTb_{c}")
                    nc.scalar.activation(n_t[:, 0:128], sn[:, 0:128], AF.Tanh)
                    nc.vector.tensor_scalar_mul(
                        zm[:], rz[:, 256:512], masks_sb[c][:, mcol:mcol + 1])
                    for o, hT_new in ((0, hTa_new), (128, hTb_new)):
                        if o:
                            nc.scalar.activation(n_t[:, o:o + 128], sn[:, o:o + 128],
                                                 AF.Tanh)
                        d_t = wpool.tile([128, 128], f16, tag=f"d_{c}{o}", name=f"d_{c}{o}")
                        nc.vector.tensor_sub(d_t[:], n_t[:, o:o + 128],
                                             h_str[c][:, o:o + 128])
                        e_t = wpool.tile([128, 128], f16, tag=f"e_{c}{o}", name=f"e_{c}{o}")
                        nc.vector.tensor_mul(e_t[:], d_t[:], zm[:, o:o + 128])
                        nc.vector.tensor_add(h_new[:, o:o + 128], e_t[:],
                                             h_str[c][:, o:o + 128])
                        nc.vector.transpose(hT_new[:], h_new[:, o:o + 128])

                    h_str[c], hTa[c], hTb[c] = h_new, hTa_new, hTb_new

            # ---- head (per chain) ---------------------------------------
            for c in range(NCH):
                ph = psum.tile([128, 512], f32, tag=f"rz_{c}", name=f"ph_{c}")
                for ki in range(KT):
                    lhsT = (hTa[c][:, 32 * ki:32 * ki + NL] if ki < 4
                            else hTb[c][:, 32 * (ki - 4):32 * (ki - 4) + NL])
                    nc.tensor.matmul(
                        ph[0:NL, 0:64],
                        lhsT,
                        p1T_sb[:, 64 * ki:64 * (ki + 1)],
                        start=(ki == 0), stop=(ki == KT - 1),
                    )
                t1s = wpool.tile([128, 64], f16, tag=f"t1s_{c}", name=f"t1s_{c}")
                nc.vector.tensor_add(t1s[0:NL, :], ph[0:NL, 0:64], p1b_sb[0:NL, :])
                t1 = wpool.tile([128, 64], f16, tag=f"t1_{c}", name=f"t1_{c}")
                nc.scalar.activation(t1[0:NL, :], t1s[0:NL, :], AF.Tanh)

                pt1 = psum.tile([128, 256], f16, tag="tp", name=f"pt1_{c}")
                nc.tensor.transpose(pt1[0:64, 0:NL], t1[0:NL, 0:64],
                                    ident_sb[0:NL, 0:NL])
                t1T = wpool.tile([64, NL], f16, tag=f"t1T_{c}", name=f"t1T_{c}")
                nc.vector.tensor_copy(t1T[:], pt1[0:64, 0:NL])

                pl = psum.tile([128, 512], f32, tag=f"rz_{c}", name=f"pl_{c}")
                nc.tensor.matmul(pl[0:NL, 0:2], t1T[:], p2T_sb[:], start=True, stop=True)
                lg = wpool.tile([128, 2], f32, tag=f"lg_{c}", name=f"lg_{c}")
                nc.vector.tensor_add(lg[0:NL, :], pl[0:NL, 0:2], p2b_sb[0:NL, :])
                nc.sync.dma_start(d_logits[NL * c:NL * (c + 1), :], lg[0:NL, :])

    nc.compile()
    return nc


# ----------------------------------------------------------------------------
# entry point
# ----------------------------------------------------------------------------

@functools.lru_cache(maxsize=2)
def _cached_program(n_steps):
    return build_program(n_steps)


def run(inputs, n_steps=T, trace=False):
    inputs = {k: np.asarray(v) for k, v in inputs.items()}
    nc = _cached_program(n_steps)
    shared = _prep_shared(inputs, n_steps)
    emb16 = np.asarray(inputs["emb"]).astype(np.float16)
    in_maps = []
    for c in range(NCORES):
        m = dict(shared)
        m.update(_prep_core(inputs, emb16, c, n_steps))
        in_maps.append(m)
    res = run_bass_kernel_spmd(nc, in_maps, list(range(NCORES)), trace=trace)
    logits = np.concatenate([res.results[c]["logits"] for c in range(NCORES)], axis=0)
    return logits, res


def kernel(**inputs) -> np.ndarray:
    logits, _ = run(inputs)
    return logits.astype(np.float32)
